# revision 9
# baseline (speedup 1.0000x reference)
"""Multi-head attention (B=4, L=2048, C=1024, H=16, HD=64) on 8 NeuronCores.

Sharding: tensor-parallel over heads — 2 heads per core. Each core computes
its heads' QKV projection, attention, and a partial output projection over
its 128 ctx channels; the host sums the 8 partial outputs.

Per-core kernel layout notes:
  - All projections/attention keep "T" layouts (channels on partitions) so
    every matmul contraction runs over the partition dim with large moving
    free dims (float32r fast path, >=256 rows).
  - Softmax skips the max-subtraction (scores are ~N(0, 1/9): exp is safe)
    and normalizes after the ctx matmul using a ones-column appended to v
    (rowsum rides along as PSUM partition 64 of the ctx accumulation).
"""

import numpy as np

import concourse.bass as bass
import concourse.mybir as mybir
import concourse.tile as tile
from concourse import bacc
from concourse.bass_utils import run_bass_kernel_spmd

B, L, C, H, HD = 4, 2048, 1024, 16, 64
NCORES = 8
HPC = H // NCORES  # heads per core = 2
F32 = mybir.dt.float32
F32R = mybir.dt.float32r

LCHUNK = 512          # token chunk for moving operands
NLC = L // LCHUNK     # 4
NKT = L // 128        # 16 k tiles per sequence
NCT = C // 128        # 8 contraction tiles for the projections


def r(ap):
    """Tag an fp32 AP as float32r for the fast PE path (same bytes)."""
    return ap.bitcast(F32R)


def build_kernel():
    nc = bacc.Bacc("TRN2", target_bir_lowering=False, debug=False,
                   num_devices=NCORES)

    xT = nc.dram_tensor("xT", [B, C, L], F32R, kind="ExternalInput")
    # wq_tiles[ci, j] = [128 c, 128 f] tile; j in (0=q both heads, 1=k, 2=v)
    wqkv = nc.dram_tensor("wqkv", [NCT, 3, 128, 128], F32R, kind="ExternalInput")
    bqkv = nc.dram_tensor("bqkv", [3, 128, 1], F32, kind="ExternalInput")
    wo = nc.dram_tensor("wo", [HPC, HD, C], F32R, kind="ExternalInput")
    bo8 = nc.dram_tensor("bo8", [128, C], F32, kind="ExternalInput")
    ident_d = nc.dram_tensor("ident_d", [128, 128], F32R, kind="ExternalInput")
    ones_d = nc.dram_tensor("ones_d", [128, HD], F32R, kind="ExternalInput")
    out = nc.dram_tensor("out", [B * L, C], F32, kind="ExternalOutput")

    with tile.TileContext(nc) as tc:
        kernel_body(nc, tc, xT, wqkv, bqkv, wo, bo8, ident_d, ones_d, out)
    nc.compile()
    return nc


def kernel_body(nc, tc, xT, wqkv, bqkv, wo, bo8, ident_d, ones_d, out):
    from contextlib import ExitStack
    ctx = ExitStack()
    with ctx:
        consts = ctx.enter_context(tc.tile_pool(name="consts", bufs=1))
        xpool = ctx.enter_context(tc.tile_pool(name="xpool", bufs=10))
        qkvpool = ctx.enter_context(tc.tile_pool(name="qkvpool", bufs=2))
        vppool = ctx.enter_context(tc.tile_pool(name="vppool", bufs=24))
        epool = ctx.enter_context(tc.tile_pool(name="epool", bufs=4))
        cpool = ctx.enter_context(tc.tile_pool(name="cpool", bufs=2))
        spool = ctx.enter_context(tc.tile_pool(name="spool", bufs=2))
        opool = ctx.enter_context(tc.tile_pool(name="opool", bufs=3))
        psum = ctx.enter_context(tc.tile_pool(name="psum", bufs=8, space="PSUM"))

        # ---- constants ----
        w_tiles = []
        for ci in range(NCT):
            row = []
            for j in range(3):
                t = consts.tile([128, 128], F32R, tag=f"w{ci}_{j}")
                nc.sync.dma_start(out=t, in_=wqkv[ci, j])
                row.append(t)
            w_tiles.append(row)
        b_tiles = []
        for j in range(3):
            t = consts.tile([128, 1], F32, tag=f"b{j}")
            nc.sync.dma_start(out=t, in_=bqkv[j])
            b_tiles.append(t)
        wo_tiles = []
        for h in range(HPC):
            t = consts.tile([HD, C], F32R, tag=f"wo{h}")
            nc.sync.dma_start(out=t, in_=wo[h])
            wo_tiles.append(t)
        bias_bc = consts.tile([128, C], F32, tag="bias_bc")
        nc.sync.dma_start(out=bias_bc, in_=bo8[:])
        ident = consts.tile([128, 128], F32R, tag="ident")
        nc.sync.dma_start(out=ident, in_=ident_d[:])
        ones_t = consts.tile([128, HD], F32R, tag="ones")
        nc.sync.dma_start(out=ones_t, in_=ones_d[:])

        for b in range(B):
            # ---- phase 1: qkv projection (channel-major) ----
            # qkvT[j] = [128 f, L]; f packs both heads (h0 at 0:64, h1 64:128)
            qkvT = [qkvpool.tile([128, L], F32R, tag=f"qkvT{j}", name=f"qkvT{j}")
                    for j in range(3)]
            for lc in range(NLC):
                ls = bass.ts(lc, LCHUNK)
                xts = []
                for ci in range(NCT):
                    xt = xpool.tile([128, LCHUNK], F32R, tag="xt")
                    nc.sync.dma_start(out=xt, in_=xT[b, bass.ts(ci, 128), ls])
                    xts.append(xt)
                for j in range(3):
                    p = psum.tile([128, LCHUNK], F32, tag="pb")
                    for ci in range(NCT):
                        nc.tensor.matmul(p, w_tiles[ci][j][:], xts[ci][:],
                                         start=(ci == 0), stop=(ci == NCT - 1))
                    # PSUM -> SBUF with per-partition bias add
                    nc.vector.tensor_scalar_add(qkvT[j][:, ls], p, b_tiles[j][:])

            # ---- phase 1b: v -> token-major tiles [128 l, v_h0|1|v_h1|1] ----
            vplus = []
            for t in range(NKT):
                tp = psum.tile([128, 128], F32R, tag="pb")
                nc.tensor.transpose(tp, qkvT[2][:, bass.ts(t, 128)], ident[:])
                vp = vppool.tile([128, 2 * HD + 2], F32R, tag="vp")
                nc.vector.tensor_copy(vp[:, 0:HD], tp[:, 0:HD])
                nc.vector.tensor_copy(vp[:, HD + 1:2 * HD + 1], tp[:, HD:2 * HD])
                nc.vector.tensor_copy(vp[:, HD:HD + 1], ones_t[:, 0:1])
                nc.vector.tensor_copy(vp[:, 2 * HD + 1:2 * HD + 2], ones_t[:, 0:1])
                vplus.append(vp)

            # ---- phase 2: attention per head / q-chunk ----
            ctxT = [cpool.tile([HD, L], F32R, tag=f"ctxT{h}", name=f"ctxT{h}")
                    for h in range(HPC)]
            for h in range(HPC):
                hb = h * HD  # partition base of this head's channels
                for qc in range(NLC):
                    qs = bass.ts(qc, LCHUNK)
                    cacc = psum.tile([HD + 1, LCHUNK], F32, tag="pb")
                    for i in range(NKT):
                        s = psum.tile([128, LCHUNK], F32, tag="pb")
                        nc.tensor.matmul(
                            s,
                            qkvT[1][hb:hb + HD, bass.ts(i, 128)],
                            qkvT[0][hb:hb + HD, qs],
                            start=True, stop=True)
                        e = epool.tile([128, LCHUNK], F32R, tag="e")
                        nc.scalar.activation(e, s,
                                             mybir.ActivationFunctionType.Exp,
                                             scale=0.125)
                        nc.tensor.matmul(
                            cacc,
                            vplus[i][:, h * (HD + 1):(h + 1) * (HD + 1)],
                            e[:],
                            start=(i == 0), stop=(i == NKT - 1))
                    # normalize: recip of rowsum (partition HD), broadcast
                    # over the HD ctx partitions via a rank-1 matmul
                    rt = spool.tile([HD + 1, LCHUNK], F32R, tag="rt")
                    with nc.allow_low_precision(reason="softmax recip in f32r"):
                        nc.vector.reciprocal(rt[HD:HD + 1, :], cacc[HD:HD + 1, :])
                    zb = psum.tile([HD, LCHUNK], F32, tag="pb")
                    nc.tensor.matmul(zb, ones_t[HD:HD + 1, 0:HD],
                                     rt[HD:HD + 1, :], start=True, stop=True)
                    zs = spool.tile([HD, LCHUNK], F32, tag="zs")
                    nc.vector.tensor_copy(zs, zb)
                    nc.vector.tensor_mul(ctxT[h][:, qs], cacc[0:HD, :], zs)

            # ---- phase 3: output projection (partial over this core's ch) ----
            for t in range(NKT):
                rows = bass.ds(b * L + t * 128, 128)
                for oc in range(C // 512):
                    os_ = bass.ts(oc, 512)
                    o = psum.tile([128, 512], F32, tag="pb")
                    for h in range(HPC):
                        nc.tensor.matmul(o, ctxT[h][:, bass.ts(t, 128)],
                                         wo_tiles[h][:, os_],
                                         start=(h == 0), stop=(h == HPC - 1))
                    ot = opool.tile([128, 512], F32, tag="ot")
                    nc.vector.tensor_add(ot, o, bias_bc[:, os_])
                    nc.sync.dma_start(out=out[rows, os_], in_=ot)


_NC_CACHE = None


def get_nc():
    global _NC_CACHE
    if _NC_CACHE is None:
        _NC_CACHE = build_kernel()
    return _NC_CACHE


def prepare_in_maps(x, W_qkv, b_qkv, W_out, b_out):
    x = np.ascontiguousarray(np.asarray(x, np.float32))
    W_qkv = np.asarray(W_qkv, np.float32)
    b_qkv = np.asarray(b_qkv, np.float32)
    W_out = np.asarray(W_out, np.float32)
    b_out = np.asarray(b_out, np.float32)

    xT = np.ascontiguousarray(x.transpose(0, 2, 1))  # [B, C, L]

    in_maps = []
    for core in range(NCORES):
        h0 = HPC * core
        # per-head channel rows in W_qkv: q = h*192..+64, k = +64, v = +128
        qrows = [np.arange(h * 192, h * 192 + 64) for h in (h0, h0 + 1)]
        krows = [q + 64 for q in qrows]
        vrows = [q + 128 for q in qrows]
        fq = np.concatenate(qrows)
        fk = np.concatenate(krows)
        fv = np.concatenate(vrows)
        # wqkv tiles: [ci, j, 128 c, 128 f]
        wt = np.empty((NCT, 3, 128, 128), np.float32)
        for j, rows in enumerate((fq, fk, fv)):
            wT = np.ascontiguousarray(W_qkv[rows].T)  # [1024 c, 128 f]
            wt[:, j] = wT.reshape(NCT, 128, 128)
        bq = np.stack([b_qkv[fq], b_qkv[fk], b_qkv[fv]])[..., None]  # [3,128,1]
        # wo[h] = W_out[:, h ctx ch].T -> [64 c, 1024 o]
        wo_c = np.stack([
            np.ascontiguousarray(W_out[:, (h0 + h) * HD:(h0 + h + 1) * HD].T)
            for h in range(HPC)
        ])
        bo8 = np.broadcast_to((b_out / NCORES)[None, :], (128, C))
        in_maps.append({
            "xT": xT,
            "wqkv": wt,
            "bqkv": np.ascontiguousarray(bq),
            "wo": wo_c,
            "bo8": np.ascontiguousarray(bo8, dtype=np.float32),
            "ident_d": np.eye(128, dtype=np.float32),
            "ones_d": np.ones((128, HD), np.float32),
        })
    return in_maps


def kernel(x, W_qkv, b_qkv, W_out, b_out):
    in_maps = prepare_in_maps(x, W_qkv, b_qkv, W_out, b_out)
    res = run_bass_kernel_spmd(get_nc(), in_maps, core_ids=list(range(NCORES)))
    acc = np.zeros((B * L, C), np.float64)
    for core_out in res.results:
        acc += core_out["out"]
    return acc.reshape(B, L, C).astype(np.float32)


if __name__ == "__main__":
    rng = np.random.default_rng(0)
    ins = {
        "x": rng.standard_normal((B, L, C), np.float32),
        "W_qkv": rng.uniform(-1 / 32, 1 / 32, (3 * C, C)).astype(np.float32),
        "b_qkv": rng.uniform(-1 / 32, 1 / 32, (3 * C,)).astype(np.float32),
        "W_out": rng.uniform(-1 / 32, 1 / 32, (C, C)).astype(np.float32),
        "b_out": rng.uniform(-1 / 32, 1 / 32, (C,)).astype(np.float32),
    }
    o = kernel(**ins)
    print(o.shape, o.dtype)


# revision 10
# speedup vs baseline: 1.0889x; 1.0889x over previous
"""Multi-head attention (B=4, L=2048, C=1024, H=16, HD=64) on 8 NeuronCores.

Sharding: tensor-parallel over heads — 2 heads per core. Each core computes
its heads' QKV projection, attention, and a partial output projection over
its 128 ctx channels; the host sums the 8 partial outputs.

Per-core kernel layout notes:
  - All projections/attention keep "T" layouts (channels on partitions) so
    every matmul contraction runs over the partition dim with 512-wide
    moving operands (float32r fast path; exp-probabilities side in bf16).
  - Softmax skips the max-subtraction (scores are ~N(0, 1/9): exp is safe)
    and normalizes after the ctx matmul using a ones-column appended to v
    (rowsum rides along as PSUM partition 64 of the ctx accumulation).
  - Matmuls of the same shape are batched into runs (PE shape switches
    measured ~0.9us each on TRN2).
"""

import numpy as np
import ml_dtypes

import concourse.bass as bass
import concourse.mybir as mybir
import concourse.tile as tile
from concourse import bacc
from concourse.bass_utils import run_bass_kernel_spmd

B, L, C, H, HD = 4, 2048, 1024, 16, 64
NCORES = 8
HPC = H // NCORES  # heads per core = 2
F32 = mybir.dt.float32
F32R = mybir.dt.float32r
BF16 = mybir.dt.bfloat16
EXP = mybir.ActivationFunctionType.Exp

LCHUNK = 512          # token chunk for moving operands
NLC = L // LCHUNK     # 4
NKT = L // 128        # 16 k tiles per sequence
NCT = C // 128        # 8 contraction tiles for the projections


def build_kernel():
    nc = bacc.Bacc("TRN2", target_bir_lowering=False, debug=False,
                   num_devices=NCORES)

    xT = nc.dram_tensor("xT", [B, C, L], F32R, kind="ExternalInput")
    # wqkv[ci, j] = [128 c, 128 f] tile; j in (0=q both heads, 1=k, 2=v)
    wqkv = nc.dram_tensor("wqkv", [NCT, 3, 128, 128], F32R, kind="ExternalInput")
    bqkv = nc.dram_tensor("bqkv", [3, 128, 1], F32, kind="ExternalInput")
    # wo2: [128 c(2 heads), 1024 o]
    wo2 = nc.dram_tensor("wo2", [128, C], F32R, kind="ExternalInput")
    bo8 = nc.dram_tensor("bo8", [128, C], F32, kind="ExternalInput")
    ident_d = nc.dram_tensor("ident_d", [128, 128], F32R, kind="ExternalInput")
    ones_d = nc.dram_tensor("ones_d", [128, HD], F32R, kind="ExternalInput")
    onesb_d = nc.dram_tensor("onesb_d", [128, 8], BF16, kind="ExternalInput")
    out = nc.dram_tensor("out", [B * L, C], F32, kind="ExternalOutput")

    with tile.TileContext(nc) as tc:
        kernel_body(nc, tc, xT, wqkv, bqkv, wo2, bo8, ident_d, ones_d,
                    onesb_d, out)
    nc.compile()
    return nc


def kernel_body(nc, tc, xT, wqkv, bqkv, wo2, bo8, ident_d, ones_d, onesb_d,
                out):
    from contextlib import ExitStack
    ctx = ExitStack()
    with ctx:
        consts = ctx.enter_context(tc.tile_pool(name="consts", bufs=1))
        xpool = ctx.enter_context(tc.tile_pool(name="xpool", bufs=10))
        qkvpool = ctx.enter_context(tc.tile_pool(name="qkvpool", bufs=2))
        vppool = ctx.enter_context(tc.tile_pool(name="vppool", bufs=24))
        epool = ctx.enter_context(tc.tile_pool(name="epool", bufs=22))
        cpool = ctx.enter_context(tc.tile_pool(name="cpool", bufs=2))
        spool = ctx.enter_context(tc.tile_pool(name="spool", bufs=3))
        opool = ctx.enter_context(tc.tile_pool(name="opool", bufs=4))
        # PSUM: s-tiles (4 banks) + cacc (2) + general (2)
        spsum = ctx.enter_context(tc.tile_pool(name="spsum", bufs=4,
                                               space="PSUM"))
        cpsum = ctx.enter_context(tc.tile_pool(name="cpsum", bufs=2,
                                               space="PSUM"))
        gpsum = ctx.enter_context(tc.tile_pool(name="gpsum", bufs=2,
                                               space="PSUM"))

        # ---- constants ----
        w_tiles = []
        for ci in range(NCT):
            row = []
            for j in range(3):
                t = consts.tile([128, 128], F32R, tag=f"w{ci}_{j}")
                nc.sync.dma_start(out=t, in_=wqkv[ci, j])
                row.append(t)
            w_tiles.append(row)
        b_tiles = []
        for j in range(3):
            t = consts.tile([128, 1], F32, tag=f"b{j}")
            nc.sync.dma_start(out=t, in_=bqkv[j])
            b_tiles.append(t)
        wo_t = consts.tile([128, C], F32R, tag="wo_t")
        nc.sync.dma_start(out=wo_t, in_=wo2[:])
        bias_bc = consts.tile([128, C], F32, tag="bias_bc")
        nc.sync.dma_start(out=bias_bc, in_=bo8[:])
        ident = consts.tile([128, 128], F32R, tag="ident")
        nc.sync.dma_start(out=ident, in_=ident_d[:])
        ones_t = consts.tile([128, HD], F32R, tag="ones")
        nc.sync.dma_start(out=ones_t, in_=ones_d[:])
        onesb = consts.tile([128, 8], BF16, tag="onesb")
        nc.sync.dma_start(out=onesb, in_=onesb_d[:])

        for b in range(B):
            # ---- phase 1: qkv projection (channel-major) ----
            # qkvT[j] = [128 f, L]; f packs both heads (h0 at 0:64, h1 64:128)
            qkvT = [qkvpool.tile([128, L], F32R, tag=f"qkvT{j}", name=f"qkvT{j}")
                    for j in range(3)]
            for lc in range(NLC):
                ls = bass.ts(lc, LCHUNK)
                xts = []
                for ci in range(NCT):
                    xt = xpool.tile([128, LCHUNK], F32R, tag="xt")
                    nc.sync.dma_start(out=xt, in_=xT[b, bass.ts(ci, 128), ls])
                    xts.append(xt)
                for j in range(3):
                    p = gpsum.tile([128, LCHUNK], F32, tag="gpb")
                    for ci in range(NCT):
                        nc.tensor.matmul(p, w_tiles[ci][j][:], xts[ci][:],
                                         start=(ci == 0), stop=(ci == NCT - 1))
                    # PSUM -> SBUF with per-partition bias add
                    nc.vector.tensor_scalar_add(qkvT[j][:, ls], p, b_tiles[j][:])

            # ---- phase 1b: v -> token-major bf16 [128 l, v_h0|1|v_h1|1] ----
            vplus = []
            for t in range(NKT):
                tp = gpsum.tile([128, 128], F32R, tag="gpb")
                nc.tensor.transpose(tp, qkvT[2][:, bass.ts(t, 128)], ident[:])
                vp = vppool.tile([128, 2 * HD + 2], BF16, tag="vp")
                nc.vector.tensor_copy(vp[:, 0:HD], tp[:, 0:HD])
                nc.vector.tensor_copy(vp[:, HD + 1:2 * HD + 1], tp[:, HD:2 * HD])
                nc.vector.tensor_copy(vp[:, HD:HD + 1], onesb[:, 0:1])
                nc.vector.tensor_copy(vp[:, 2 * HD + 1:2 * HD + 2], onesb[:, 0:1])
                vplus.append(vp)

            # ---- phase 2: attention per head / q-chunk ----
            # ctxT2 packs both heads: rows 0:64 = h0 channels, 64:128 = h1
            ctxT2 = cpool.tile([128, L], F32R, tag="ctxT2", name="ctxT2")
            for h in range(HPC):
                hb = h * HD  # partition base of this head's q/k channels
                for qc in range(NLC):
                    qs = bass.ts(qc, LCHUNK)
                    cacc = cpsum.tile([HD + 1, LCHUNK], F32, tag="cpb")
                    evec = []
                    for i in range(NKT):
                        s = spsum.tile([128, LCHUNK], F32, tag="spb")
                        nc.tensor.matmul(
                            s,
                            qkvT[1][hb:hb + HD, bass.ts(i, 128)],
                            qkvT[0][hb:hb + HD, qs],
                            start=True, stop=True)
                        e = epool.tile([128, LCHUNK], BF16, tag="e")
                        nc.scalar.activation(e, s, EXP, scale=0.125)
                        evec.append(e)
                    for i in range(NKT):
                        nc.tensor.matmul(
                            cacc,
                            vplus[i][:, h * (HD + 1):(h + 1) * (HD + 1)],
                            evec[i][:],
                            start=(i == 0), stop=(i == NKT - 1))
                    # normalize: recip of rowsum (partition HD), broadcast
                    # over the HD ctx partitions via a rank-1 matmul
                    rt = spool.tile([HD + 1, LCHUNK], F32R, tag="rt")
                    with nc.allow_low_precision(reason="softmax recip"):
                        nc.vector.reciprocal(rt[HD:HD + 1, :],
                                             cacc[HD:HD + 1, :])
                    zb = gpsum.tile([HD, LCHUNK], F32, tag="gpb")
                    nc.tensor.matmul(zb, ones_t[HD:HD + 1, 0:HD],
                                     rt[HD:HD + 1, :], start=True, stop=True)
                    zs = spool.tile([HD, LCHUNK], F32, tag="zs")
                    nc.vector.tensor_copy(zs, zb)
                    nc.vector.tensor_mul(ctxT2[hb:hb + HD, qs],
                                         cacc[0:HD, :], zs)

            # ---- phase 3: output projection (both heads, K=128) ----
            for t in range(NKT):
                rows = bass.ds(b * L + t * 128, 128)
                for oc in range(C // 512):
                    os_ = bass.ts(oc, 512)
                    o = gpsum.tile([128, 512], F32, tag="gpb")
                    nc.tensor.matmul(o, ctxT2[:, bass.ts(t, 128)],
                                     wo_t[:, os_], start=True, stop=True)
                    ot = opool.tile([128, 512], F32, tag="ot")
                    nc.vector.tensor_add(ot, o, bias_bc[:, os_])
                    nc.sync.dma_start(out=out[rows, os_], in_=ot)


_NC_CACHE = None


def get_nc():
    global _NC_CACHE
    if _NC_CACHE is None:
        _NC_CACHE = build_kernel()
    return _NC_CACHE


def prepare_in_maps(x, W_qkv, b_qkv, W_out, b_out):
    x = np.ascontiguousarray(np.asarray(x, np.float32))
    W_qkv = np.asarray(W_qkv, np.float32)
    b_qkv = np.asarray(b_qkv, np.float32)
    W_out = np.asarray(W_out, np.float32)
    b_out = np.asarray(b_out, np.float32)

    xT = np.ascontiguousarray(x.transpose(0, 2, 1))  # [B, C, L]

    in_maps = []
    for core in range(NCORES):
        h0 = HPC * core
        # per-head channel rows in W_qkv: q = h*192..+64, k = +64, v = +128
        qrows = [np.arange(h * 192, h * 192 + 64) for h in (h0, h0 + 1)]
        krows = [q + 64 for q in qrows]
        vrows = [q + 128 for q in qrows]
        fq = np.concatenate(qrows)
        fk = np.concatenate(krows)
        fv = np.concatenate(vrows)
        # wqkv tiles: [ci, j, 128 c, 128 f]
        wt = np.empty((NCT, 3, 128, 128), np.float32)
        for j, rows in enumerate((fq, fk, fv)):
            wT = np.ascontiguousarray(W_qkv[rows].T)  # [1024 c, 128 f]
            wt[:, j] = wT.reshape(NCT, 128, 128)
        bq = np.stack([b_qkv[fq], b_qkv[fk], b_qkv[fv]])[..., None]  # [3,128,1]
        # wo2 = [128 c, 1024 o]: rows 0:64 h0 ctx channels, 64:128 h1
        wo2 = np.concatenate([
            np.ascontiguousarray(W_out[:, (h0 + h) * HD:(h0 + h + 1) * HD].T)
            for h in range(HPC)
        ], axis=0)
        bo8 = np.broadcast_to((b_out / NCORES)[None, :], (128, C))
        in_maps.append({
            "xT": xT,
            "wqkv": wt,
            "bqkv": np.ascontiguousarray(bq),
            "wo2": np.ascontiguousarray(wo2, dtype=np.float32),
            "bo8": np.ascontiguousarray(bo8, dtype=np.float32),
            "ident_d": np.eye(128, dtype=np.float32),
            "ones_d": np.ones((128, HD), np.float32),
            "onesb_d": np.ones((128, 8), ml_dtypes.bfloat16),
        })
    return in_maps


def kernel(x, W_qkv, b_qkv, W_out, b_out):
    in_maps = prepare_in_maps(x, W_qkv, b_qkv, W_out, b_out)
    res = run_bass_kernel_spmd(get_nc(), in_maps, core_ids=list(range(NCORES)))
    acc = np.zeros((B * L, C), np.float64)
    for core_out in res.results:
        acc += core_out["out"]
    return acc.reshape(B, L, C).astype(np.float32)


if __name__ == "__main__":
    rng = np.random.default_rng(0)
    ins = {
        "x": rng.standard_normal((B, L, C)).astype(np.float32),
        "W_qkv": rng.uniform(-1 / 32, 1 / 32, (3 * C, C)).astype(np.float32),
        "b_qkv": rng.uniform(-1 / 32, 1 / 32, (3 * C,)).astype(np.float32),
        "W_out": rng.uniform(-1 / 32, 1 / 32, (C, C)).astype(np.float32),
        "b_out": rng.uniform(-1 / 32, 1 / 32, (C,)).astype(np.float32),
    }
    o = kernel(**ins)
    print(o.shape, o.dtype)


# revision 12
# speedup vs baseline: 1.2710x; 1.1673x over previous
"""Multi-head attention (B=4, L=2048, C=1024, H=16, HD=64) on 8 NeuronCores.

Sharding: tensor-parallel over heads — 2 heads per core. Each core computes
its heads' QKV projection, attention, and a partial output projection over
its 128 ctx channels; the host sums the 8 partial outputs.

Per-core kernel layout notes:
  - All projections/attention keep "T" layouts (channels on partitions) so
    every matmul contraction runs over the partition dim with 512-wide
    moving operands (float32r fast path; exp-probabilities side in bf16).
  - Softmax skips the max-subtraction (scores are ~N(0, 1/9): exp is safe)
    and normalizes after the ctx matmul using a ones-column appended to v
    (rowsum rides along as PSUM partition 64 of the ctx accumulation).
  - Matmuls of the same shape are batched into runs (PE shape switches
    measured ~0.9us each on TRN2).
"""

import numpy as np
import ml_dtypes

import concourse.bass as bass
import concourse.mybir as mybir
import concourse.tile as tile
from concourse import bacc
from concourse.bass_utils import run_bass_kernel_spmd

B, L, C, H, HD = 4, 2048, 1024, 16, 64
NCORES = 8
HPC = H // NCORES  # heads per core = 2
F32 = mybir.dt.float32
F32R = mybir.dt.float32r
BF16 = mybir.dt.bfloat16
EXP = mybir.ActivationFunctionType.Exp

LCHUNK = 512          # token chunk for moving operands
NLC = L // LCHUNK     # 4
NKT = L // 128        # 16 k tiles per sequence
NCT = C // 128        # 8 contraction tiles for the projections


def build_kernel():
    nc = bacc.Bacc("TRN2", target_bir_lowering=False, debug=False,
                   num_devices=NCORES)

    xT = nc.dram_tensor("xT", [B, C, L], F32R, kind="ExternalInput")
    # wqkv[ci, j] = [128 c, 128 f] tile; j in (0=q both heads, 1=k, 2=v)
    wqkv = nc.dram_tensor("wqkv", [NCT, 3, 128, 128], F32R, kind="ExternalInput")
    bqkv = nc.dram_tensor("bqkv", [3, 128, 1], F32, kind="ExternalInput")
    # wo2: [128 c(2 heads), 1024 o]
    wo2 = nc.dram_tensor("wo2", [128, C], F32R, kind="ExternalInput")
    bo8 = nc.dram_tensor("bo8", [128, C], F32, kind="ExternalInput")
    ident_d = nc.dram_tensor("ident_d", [128, 128], F32R, kind="ExternalInput")
    ones_d = nc.dram_tensor("ones_d", [128, HD], F32R, kind="ExternalInput")
    onesb_d = nc.dram_tensor("onesb_d", [128, 8], BF16, kind="ExternalInput")
    out = nc.dram_tensor("out", [B * L, C], F32, kind="ExternalOutput")

    with tile.TileContext(nc) as tc:
        kernel_body(nc, tc, xT, wqkv, bqkv, wo2, bo8, ident_d, ones_d,
                    onesb_d, out)
    nc.compile()
    return nc


def kernel_body(nc, tc, xT, wqkv, bqkv, wo2, bo8, ident_d, ones_d, onesb_d,
                out):
    from contextlib import ExitStack
    ctx = ExitStack()
    with ctx:
        consts = ctx.enter_context(tc.tile_pool(name="consts", bufs=1))
        xpool = ctx.enter_context(tc.tile_pool(name="xpool", bufs=16))
        qkvpool = ctx.enter_context(tc.tile_pool(name="qkvpool", bufs=2))
        vppool = ctx.enter_context(tc.tile_pool(name="vppool", bufs=24))
        epool = ctx.enter_context(tc.tile_pool(name="epool", bufs=20))
        cpool = ctx.enter_context(tc.tile_pool(name="cpool", bufs=2))
        spool = ctx.enter_context(tc.tile_pool(name="spool", bufs=3))
        opool = ctx.enter_context(tc.tile_pool(name="opool", bufs=4))
        # PSUM banks: s-tiles 2x2 + cacc 2 + general 2 = 8
        spsum = ctx.enter_context(tc.tile_pool(name="spsum", bufs=2,
                                               space="PSUM"))
        cpsum = ctx.enter_context(tc.tile_pool(name="cpsum", bufs=2,
                                               space="PSUM"))
        gpsum = ctx.enter_context(tc.tile_pool(name="gpsum", bufs=2,
                                               space="PSUM"))

        # ---- constants ----
        w_tiles = []
        for ci in range(NCT):
            row = []
            for j in range(3):
                t = consts.tile([128, 128], F32R, tag=f"w{ci}_{j}")
                nc.sync.dma_start(out=t, in_=wqkv[ci, j])
                row.append(t)
            w_tiles.append(row)
        b_tiles = []
        for j in range(3):
            t = consts.tile([128, 1], F32, tag=f"b{j}")
            nc.sync.dma_start(out=t, in_=bqkv[j])
            b_tiles.append(t)
        wo_t = consts.tile([128, C], F32R, tag="wo_t")
        nc.sync.dma_start(out=wo_t, in_=wo2[:])
        bias_bc = consts.tile([128, C], F32, tag="bias_bc")
        nc.sync.dma_start(out=bias_bc, in_=bo8[:])
        ident = consts.tile([128, 128], F32R, tag="ident")
        nc.sync.dma_start(out=ident, in_=ident_d[:])
        ones_t = consts.tile([128, HD], F32R, tag="ones")
        nc.sync.dma_start(out=ones_t, in_=ones_d[:])
        onesb = consts.tile([128, 8], BF16, tag="onesb")
        nc.sync.dma_start(out=onesb, in_=onesb_d[:])

        for b in range(B):
            # ---- phase 1: qkv projection (channel-major) ----
            # qkvT[j] = [128 f, L]; f packs both heads (h0 at 0:64, h1 64:128)
            qkvT = [qkvpool.tile([128, L], F32R, tag=f"qkvT{j}", name=f"qkvT{j}")
                    for j in range(3)]
            for lc in range(NLC):
                ls = bass.ts(lc, LCHUNK)
                xts = []
                for ci in range(NCT):
                    xt = xpool.tile([128, LCHUNK], F32R, tag="xt")
                    nc.sync.dma_start(out=xt, in_=xT[b, bass.ts(ci, 128), ls])
                    xts.append(xt)
                for j in range(3):
                    p = gpsum.tile([128, LCHUNK], F32, tag="gpb")
                    for ci in range(NCT):
                        nc.tensor.matmul(p, w_tiles[ci][j][:], xts[ci][:],
                                         start=(ci == 0), stop=(ci == NCT - 1))
                    # PSUM -> SBUF with per-partition bias add
                    nc.vector.tensor_scalar_add(qkvT[j][:, ls], p, b_tiles[j][:])

            # ---- phase 1b: v -> token-major bf16 [128 l, v_h0|1|v_h1|1] ----
            vplus = []
            for t in range(NKT):
                tp = gpsum.tile([128, 128], F32R, tag="gpb")
                nc.tensor.transpose(tp, qkvT[2][:, bass.ts(t, 128)], ident[:])
                vp = vppool.tile([128, 2 * HD + 2], BF16, tag="vp")
                nc.vector.tensor_copy(vp[:, 0:HD], tp[:, 0:HD])
                nc.vector.tensor_copy(vp[:, HD + 1:2 * HD + 1], tp[:, HD:2 * HD])
                nc.vector.tensor_copy(vp[:, HD:HD + 1], onesb[:, 0:1])
                nc.vector.tensor_copy(vp[:, 2 * HD + 1:2 * HD + 2], onesb[:, 0:1])
                vplus.append(vp)

            # ---- phase 2: attention per head / q-chunk ----
            # ctxT2 packs both heads: rows 0:64 = h0 channels, 64:128 = h1
            ctxT2 = cpool.tile([128, L], F32R, tag="ctxT2", name="ctxT2")
            for h in range(HPC):
                hb = h * HD  # partition base of this head's q/k channels
                vsl = slice(h * (HD + 1), (h + 1) * (HD + 1))
                for qc in range(2):  # q processed in chunks of 1024
                    q0 = qc * 1024
                    caccs = [cpsum.tile([HD + 1, LCHUNK], F32, tag="cpb",
                                        name=f"cacc{half}")
                             for half in range(2)]
                    evec = []
                    for i in range(NKT):
                        s = spsum.tile([128, 2 * LCHUNK], F32, tag="spb")
                        for half in range(2):
                            nc.tensor.matmul(
                                s[:, bass.ts(half, LCHUNK)],
                                qkvT[1][hb:hb + HD, bass.ts(i, 128)],
                                qkvT[0][hb:hb + HD,
                                        bass.ds(q0 + half * LCHUNK, LCHUNK)],
                                start=True, stop=True)
                        e = epool.tile([128, 2 * LCHUNK], BF16, tag="e")
                        nc.scalar.activation(e, s, EXP, scale=0.125)
                        evec.append(e)
                    for i in range(NKT):
                        for half in range(2):
                            nc.tensor.matmul(
                                caccs[half],
                                vplus[i][:, vsl],
                                evec[i][:, bass.ts(half, LCHUNK)],
                                start=(i == 0), stop=(i == NKT - 1))
                    # normalize: recip of rowsum (partition HD), broadcast
                    # over the HD ctx partitions via a rank-1 matmul
                    for half in range(2):
                        cacc = caccs[half]
                        qs = bass.ds(q0 + half * LCHUNK, LCHUNK)
                        rt = spool.tile([HD + 1, LCHUNK], F32R, tag="rt")
                        with nc.allow_low_precision(reason="softmax recip"):
                            nc.vector.reciprocal(rt[HD:HD + 1, :],
                                                 cacc[HD:HD + 1, :])
                        zb = gpsum.tile([HD, LCHUNK], F32, tag="gpb")
                        nc.tensor.matmul(zb, ones_t[HD:HD + 1, 0:HD],
                                         rt[HD:HD + 1, :],
                                         start=True, stop=True)
                        zs = spool.tile([HD, LCHUNK], F32, tag="zs")
                        nc.vector.tensor_copy(zs, zb)
                        nc.vector.tensor_mul(ctxT2[hb:hb + HD, qs],
                                             cacc[0:HD, :], zs)

            # ---- phase 3: output projection (both heads, K=128) ----
            for t in range(NKT):
                rows = bass.ds(b * L + t * 128, 128)
                for oc in range(C // 512):
                    os_ = bass.ts(oc, 512)
                    o = gpsum.tile([128, 512], F32, tag="gpb")
                    nc.tensor.matmul(o, ctxT2[:, bass.ts(t, 128)],
                                     wo_t[:, os_], start=True, stop=True)
                    ot = opool.tile([128, 512], F32, tag="ot")
                    nc.vector.tensor_add(ot, o, bias_bc[:, os_])
                    nc.sync.dma_start(out=out[rows, os_], in_=ot)


_NC_CACHE = None


def get_nc():
    global _NC_CACHE
    if _NC_CACHE is None:
        _NC_CACHE = build_kernel()
    return _NC_CACHE


def prepare_in_maps(x, W_qkv, b_qkv, W_out, b_out):
    x = np.ascontiguousarray(np.asarray(x, np.float32))
    W_qkv = np.asarray(W_qkv, np.float32)
    b_qkv = np.asarray(b_qkv, np.float32)
    W_out = np.asarray(W_out, np.float32)
    b_out = np.asarray(b_out, np.float32)

    xT = np.ascontiguousarray(x.transpose(0, 2, 1))  # [B, C, L]

    in_maps = []
    for core in range(NCORES):
        h0 = HPC * core
        # per-head channel rows in W_qkv: q = h*192..+64, k = +64, v = +128
        qrows = [np.arange(h * 192, h * 192 + 64) for h in (h0, h0 + 1)]
        krows = [q + 64 for q in qrows]
        vrows = [q + 128 for q in qrows]
        fq = np.concatenate(qrows)
        fk = np.concatenate(krows)
        fv = np.concatenate(vrows)
        # wqkv tiles: [ci, j, 128 c, 128 f]
        wt = np.empty((NCT, 3, 128, 128), np.float32)
        for j, rows in enumerate((fq, fk, fv)):
            wT = np.ascontiguousarray(W_qkv[rows].T)  # [1024 c, 128 f]
            wt[:, j] = wT.reshape(NCT, 128, 128)
        bq = np.stack([b_qkv[fq], b_qkv[fk], b_qkv[fv]])[..., None]  # [3,128,1]
        # wo2 = [128 c, 1024 o]: rows 0:64 h0 ctx channels, 64:128 h1
        wo2 = np.concatenate([
            np.ascontiguousarray(W_out[:, (h0 + h) * HD:(h0 + h + 1) * HD].T)
            for h in range(HPC)
        ], axis=0)
        bo8 = np.broadcast_to((b_out / NCORES)[None, :], (128, C))
        in_maps.append({
            "xT": xT,
            "wqkv": wt,
            "bqkv": np.ascontiguousarray(bq),
            "wo2": np.ascontiguousarray(wo2, dtype=np.float32),
            "bo8": np.ascontiguousarray(bo8, dtype=np.float32),
            "ident_d": np.eye(128, dtype=np.float32),
            "ones_d": np.ones((128, HD), np.float32),
            "onesb_d": np.ones((128, 8), ml_dtypes.bfloat16),
        })
    return in_maps


def kernel(x, W_qkv, b_qkv, W_out, b_out):
    in_maps = prepare_in_maps(x, W_qkv, b_qkv, W_out, b_out)
    res = run_bass_kernel_spmd(get_nc(), in_maps, core_ids=list(range(NCORES)))
    acc = np.zeros((B * L, C), np.float64)
    for core_out in res.results:
        acc += core_out["out"]
    return acc.reshape(B, L, C).astype(np.float32)


if __name__ == "__main__":
    rng = np.random.default_rng(0)
    ins = {
        "x": rng.standard_normal((B, L, C)).astype(np.float32),
        "W_qkv": rng.uniform(-1 / 32, 1 / 32, (3 * C, C)).astype(np.float32),
        "b_qkv": rng.uniform(-1 / 32, 1 / 32, (3 * C,)).astype(np.float32),
        "W_out": rng.uniform(-1 / 32, 1 / 32, (C, C)).astype(np.float32),
        "b_out": rng.uniform(-1 / 32, 1 / 32, (C,)).astype(np.float32),
    }
    o = kernel(**ins)
    print(o.shape, o.dtype)


# revision 13
# speedup vs baseline: 1.3115x; 1.0318x over previous
"""Multi-head attention (B=4, L=2048, C=1024, H=16, HD=64) on 8 NeuronCores.

Sharding: tensor-parallel over heads — 2 heads per core. Each core computes
its heads' QKV projection, attention, and a partial output projection over
its 128 ctx channels; the host sums the 8 partial outputs.

Per-core kernel layout notes:
  - All projections/attention keep "T" layouts (channels on partitions) so
    every matmul contraction runs over the partition dim with 512-wide
    moving operands (float32r fast path; exp-probabilities side in bf16).
  - Softmax skips the max-subtraction (scores are ~N(0, 1/9): exp is safe)
    and normalizes after the ctx matmul using a ones-column appended to v
    (rowsum rides along as PSUM partition 64 of the ctx accumulation).
  - Matmuls of the same shape are batched into runs (PE shape switches
    measured ~0.9us each on TRN2).
"""

import numpy as np
import ml_dtypes

import concourse.bass as bass
import concourse.mybir as mybir
import concourse.tile as tile
from concourse import bacc
from concourse.bass_utils import run_bass_kernel_spmd

B, L, C, H, HD = 4, 2048, 1024, 16, 64
NCORES = 8
HPC = H // NCORES  # heads per core = 2
F32 = mybir.dt.float32
F32R = mybir.dt.float32r
BF16 = mybir.dt.bfloat16
EXP = mybir.ActivationFunctionType.Exp

LCHUNK = 512          # token chunk for moving operands
NLC = L // LCHUNK     # 4
NKT = L // 128        # 16 k tiles per sequence
NCT = C // 128        # 8 contraction tiles for the projections


def build_kernel():
    nc = bacc.Bacc("TRN2", target_bir_lowering=False, debug=False,
                   num_devices=NCORES)

    xT = nc.dram_tensor("xT", [B, C, L], F32R, kind="ExternalInput")
    # wqkv[ci, j] = [128 c, 128 f] tile; j in (0=q both heads, 1=k, 2=v)
    wqkv = nc.dram_tensor("wqkv", [NCT, 3, 128, 128], F32R, kind="ExternalInput")
    bqkv = nc.dram_tensor("bqkv", [3, 128, 1], F32, kind="ExternalInput")
    # wo2: [128 c(2 heads), 1024 o]
    wo2 = nc.dram_tensor("wo2", [128, C], F32R, kind="ExternalInput")
    bo8 = nc.dram_tensor("bo8", [128, C], F32, kind="ExternalInput")
    ident_d = nc.dram_tensor("ident_d", [128, 128], F32R, kind="ExternalInput")
    onesb_d = nc.dram_tensor("onesb_d", [128, 8], BF16, kind="ExternalInput")
    out = nc.dram_tensor("out", [B * L, C], F32, kind="ExternalOutput")

    with tile.TileContext(nc) as tc:
        kernel_body(nc, tc, xT, wqkv, bqkv, wo2, bo8, ident_d,
                    onesb_d, out)
    nc.compile()
    return nc


def kernel_body(nc, tc, xT, wqkv, bqkv, wo2, bo8, ident_d, onesb_d,
                out):
    from contextlib import ExitStack
    ctx = ExitStack()
    with ctx:
        consts = ctx.enter_context(tc.tile_pool(name="consts", bufs=1))
        xpool = ctx.enter_context(tc.tile_pool(name="xpool", bufs=16))
        qkvpool = ctx.enter_context(tc.tile_pool(name="qkvpool", bufs=2))
        vppool = ctx.enter_context(tc.tile_pool(name="vppool", bufs=24))
        epool = ctx.enter_context(tc.tile_pool(name="epool", bufs=20))
        cpool = ctx.enter_context(tc.tile_pool(name="cpool", bufs=2))
        spool = ctx.enter_context(tc.tile_pool(name="spool", bufs=3))
        opool = ctx.enter_context(tc.tile_pool(name="opool", bufs=4))
        # PSUM banks: s-tiles 2x2 + cacc 2 + general 2 = 8
        spsum = ctx.enter_context(tc.tile_pool(name="spsum", bufs=2,
                                               space="PSUM"))
        cpsum = ctx.enter_context(tc.tile_pool(name="cpsum", bufs=2,
                                               space="PSUM"))
        gpsum = ctx.enter_context(tc.tile_pool(name="gpsum", bufs=2,
                                               space="PSUM"))

        # ---- constants ----
        w_tiles = []
        for ci in range(NCT):
            row = []
            for j in range(3):
                t = consts.tile([128, 128], F32R, tag=f"w{ci}_{j}")
                nc.sync.dma_start(out=t, in_=wqkv[ci, j])
                row.append(t)
            w_tiles.append(row)
        b_tiles = []
        for j in range(3):
            t = consts.tile([128, 1], F32, tag=f"b{j}")
            nc.sync.dma_start(out=t, in_=bqkv[j])
            b_tiles.append(t)
        wo_t = consts.tile([128, C], F32R, tag="wo_t")
        nc.sync.dma_start(out=wo_t, in_=wo2[:])
        bias_bc = consts.tile([128, C], F32, tag="bias_bc")
        nc.sync.dma_start(out=bias_bc, in_=bo8[:])
        ident = consts.tile([128, 128], F32R, tag="ident")
        nc.sync.dma_start(out=ident, in_=ident_d[:])
        onesb = consts.tile([128, 8], BF16, tag="onesb")
        nc.sync.dma_start(out=onesb, in_=onesb_d[:])

        for b in range(B):
            # ---- phase 1: qkv projection (channel-major) ----
            # qkvT[j] = [128 f, L]; f packs both heads (h0 at 0:64, h1 64:128)
            qkvT = [qkvpool.tile([128, L], F32R, tag=f"qkvT{j}", name=f"qkvT{j}")
                    for j in range(3)]
            for lc in range(NLC):
                ls = bass.ts(lc, LCHUNK)
                xts = []
                for ci in range(NCT):
                    xt = xpool.tile([128, LCHUNK], F32R, tag="xt")
                    nc.sync.dma_start(out=xt, in_=xT[b, bass.ts(ci, 128), ls])
                    xts.append(xt)
                for j in range(3):
                    p = gpsum.tile([128, LCHUNK], F32, tag="gpb")
                    for ci in range(NCT):
                        nc.tensor.matmul(p, w_tiles[ci][j][:], xts[ci][:],
                                         start=(ci == 0), stop=(ci == NCT - 1))
                    # PSUM -> SBUF with per-partition bias add
                    nc.vector.tensor_scalar_add(qkvT[j][:, ls], p, b_tiles[j][:])

            # ---- phase 1b: v -> token-major bf16 [128 l, v_h0|1|v_h1|1] ----
            vplus = []
            for t in range(NKT):
                tp = gpsum.tile([128, 128], F32R, tag="gpb")
                nc.tensor.transpose(tp, qkvT[2][:, bass.ts(t, 128)], ident[:])
                vp = vppool.tile([128, 2 * HD + 2], BF16, tag="vp")
                nc.vector.tensor_copy(vp[:, 0:HD], tp[:, 0:HD])
                nc.vector.tensor_copy(vp[:, HD + 1:2 * HD + 1], tp[:, HD:2 * HD])
                nc.vector.tensor_copy(vp[:, HD:HD + 1], onesb[:, 0:1])
                nc.vector.tensor_copy(vp[:, 2 * HD + 1:2 * HD + 2], onesb[:, 0:1])
                vplus.append(vp)

            # ---- phase 2: attention per head / q-chunk ----
            # ctxT2 packs both heads: rows 0:64 = h0 channels, 64:128 = h1
            ctxT2 = cpool.tile([128, L], F32R, tag="ctxT2", name="ctxT2")
            for h in range(HPC):
                hb = h * HD  # partition base of this head's q/k channels
                vsl = slice(h * (HD + 1), (h + 1) * (HD + 1))
                for qc in range(2):  # q processed in chunks of 1024
                    q0 = qc * 1024
                    caccs = [cpsum.tile([HD + 1, LCHUNK], F32, tag="cpb",
                                        name=f"cacc{half}")
                             for half in range(2)]
                    evec = []
                    for i in range(NKT):
                        s = spsum.tile([128, 2 * LCHUNK], F32, tag="spb")
                        for half in range(2):
                            nc.tensor.matmul(
                                s[:, bass.ts(half, LCHUNK)],
                                qkvT[1][hb:hb + HD, bass.ts(i, 128)],
                                qkvT[0][hb:hb + HD,
                                        bass.ds(q0 + half * LCHUNK, LCHUNK)],
                                start=True, stop=True)
                        e = epool.tile([128, 2 * LCHUNK], BF16, tag="e")
                        nc.scalar.activation(e, s, EXP, scale=0.125)
                        evec.append(e)
                    for i in range(NKT):
                        for half in range(2):
                            nc.tensor.matmul(
                                caccs[half],
                                vplus[i][:, vsl],
                                evec[i][:, bass.ts(half, LCHUNK)],
                                start=(i == 0), stop=(i == NKT - 1))
                    # normalize: recip of rowsum (partition HD lands on
                    # partition 0 via DVE), broadcast over the HD ctx
                    # partitions on GpSimd, multiply on DVE
                    for half in range(2):
                        cacc = caccs[half]
                        qs = bass.ds(q0 + half * LCHUNK, LCHUNK)
                        rt0 = spool.tile([1, LCHUNK], F32, tag="rt0")
                        nc.vector.reciprocal(rt0[0:1, :], cacc[HD:HD + 1, :])
                        zs = spool.tile([HD, LCHUNK], F32, tag="zs")
                        nc.gpsimd.partition_broadcast(zs[0:HD, :], rt0[0:1, :])
                        nc.vector.tensor_mul(ctxT2[hb:hb + HD, qs],
                                             cacc[0:HD, :], zs)

            # ---- phase 3: output projection (both heads, K=128) ----
            for t in range(NKT):
                rows = bass.ds(b * L + t * 128, 128)
                for oc in range(C // 512):
                    os_ = bass.ts(oc, 512)
                    o = gpsum.tile([128, 512], F32, tag="gpb")
                    nc.tensor.matmul(o, ctxT2[:, bass.ts(t, 128)],
                                     wo_t[:, os_], start=True, stop=True)
                    ot = opool.tile([128, 512], F32, tag="ot")
                    nc.vector.tensor_add(ot, o, bias_bc[:, os_])
                    nc.sync.dma_start(out=out[rows, os_], in_=ot)


_NC_CACHE = None


def get_nc():
    global _NC_CACHE
    if _NC_CACHE is None:
        _NC_CACHE = build_kernel()
    return _NC_CACHE


def prepare_in_maps(x, W_qkv, b_qkv, W_out, b_out):
    x = np.ascontiguousarray(np.asarray(x, np.float32))
    W_qkv = np.asarray(W_qkv, np.float32)
    b_qkv = np.asarray(b_qkv, np.float32)
    W_out = np.asarray(W_out, np.float32)
    b_out = np.asarray(b_out, np.float32)

    xT = np.ascontiguousarray(x.transpose(0, 2, 1))  # [B, C, L]

    in_maps = []
    for core in range(NCORES):
        h0 = HPC * core
        # per-head channel rows in W_qkv: q = h*192..+64, k = +64, v = +128
        qrows = [np.arange(h * 192, h * 192 + 64) for h in (h0, h0 + 1)]
        krows = [q + 64 for q in qrows]
        vrows = [q + 128 for q in qrows]
        fq = np.concatenate(qrows)
        fk = np.concatenate(krows)
        fv = np.concatenate(vrows)
        # wqkv tiles: [ci, j, 128 c, 128 f]
        wt = np.empty((NCT, 3, 128, 128), np.float32)
        for j, rows in enumerate((fq, fk, fv)):
            wT = np.ascontiguousarray(W_qkv[rows].T)  # [1024 c, 128 f]
            wt[:, j] = wT.reshape(NCT, 128, 128)
        bq = np.stack([b_qkv[fq], b_qkv[fk], b_qkv[fv]])[..., None]  # [3,128,1]
        # wo2 = [128 c, 1024 o]: rows 0:64 h0 ctx channels, 64:128 h1
        wo2 = np.concatenate([
            np.ascontiguousarray(W_out[:, (h0 + h) * HD:(h0 + h + 1) * HD].T)
            for h in range(HPC)
        ], axis=0)
        bo8 = np.broadcast_to((b_out / NCORES)[None, :], (128, C))
        in_maps.append({
            "xT": xT,
            "wqkv": wt,
            "bqkv": np.ascontiguousarray(bq),
            "wo2": np.ascontiguousarray(wo2, dtype=np.float32),
            "bo8": np.ascontiguousarray(bo8, dtype=np.float32),
            "ident_d": np.eye(128, dtype=np.float32),
            "onesb_d": np.ones((128, 8), ml_dtypes.bfloat16),
        })
    return in_maps


def kernel(x, W_qkv, b_qkv, W_out, b_out):
    in_maps = prepare_in_maps(x, W_qkv, b_qkv, W_out, b_out)
    res = run_bass_kernel_spmd(get_nc(), in_maps, core_ids=list(range(NCORES)))
    acc = np.zeros((B * L, C), np.float64)
    for core_out in res.results:
        acc += core_out["out"]
    return acc.reshape(B, L, C).astype(np.float32)


if __name__ == "__main__":
    rng = np.random.default_rng(0)
    ins = {
        "x": rng.standard_normal((B, L, C)).astype(np.float32),
        "W_qkv": rng.uniform(-1 / 32, 1 / 32, (3 * C, C)).astype(np.float32),
        "b_qkv": rng.uniform(-1 / 32, 1 / 32, (3 * C,)).astype(np.float32),
        "W_out": rng.uniform(-1 / 32, 1 / 32, (C, C)).astype(np.float32),
        "b_out": rng.uniform(-1 / 32, 1 / 32, (C,)).astype(np.float32),
    }
    o = kernel(**ins)
    print(o.shape, o.dtype)


# revision 14
# speedup vs baseline: 1.6556x; 1.2624x over previous
"""Multi-head attention (B=4, L=2048, C=1024, H=16, HD=64) on 8 NeuronCores.

Sharding: tensor-parallel over heads — 2 heads per core. Each core computes
its heads' QKV projection, attention, and a partial output projection over
its 128 ctx channels; the host sums the 8 partial outputs.

Per-core kernel layout notes:
  - All projections/attention keep "T" layouts (channels on partitions) so
    every matmul contraction runs over the partition dim with 512-wide
    moving operands (float32r fast path; exp-probabilities side in bf16).
  - Softmax skips the max-subtraction (scores are ~N(0, 1/9): exp is safe)
    and normalizes after the ctx matmul using a ones-column appended to v
    (rowsum rides along as PSUM partition 64 of the ctx accumulation).
  - Matmuls of the same shape are batched into runs (PE shape switches
    measured ~0.9us each on TRN2).
"""

import numpy as np
import ml_dtypes

import concourse.bass as bass
import concourse.mybir as mybir
import concourse.tile as tile
from concourse import bacc
from concourse.bass_utils import run_bass_kernel_spmd

B, L, C, H, HD = 4, 2048, 1024, 16, 64
NCORES = 8
HPC = H // NCORES  # heads per core = 2
F32 = mybir.dt.float32
F32R = mybir.dt.float32r
BF16 = mybir.dt.bfloat16
EXP = mybir.ActivationFunctionType.Exp

LCHUNK = 512          # token chunk for moving operands
NLC = L // LCHUNK     # 4
NKT = L // 128        # 16 k tiles per sequence
NCT = C // 128        # 8 contraction tiles for the projections


def build_kernel():
    nc = bacc.Bacc("TRN2", target_bir_lowering=False, debug=False,
                   num_devices=NCORES)

    xT = nc.dram_tensor("xT", [B, C, L], F32R, kind="ExternalInput")
    # wqkv[ci, j] = [128 c, 128 f] tile; j in (0=q both heads, 1=k, 2=v)
    wqkv = nc.dram_tensor("wqkv", [NCT, 3, 128, 128], F32R, kind="ExternalInput")
    bqkv = nc.dram_tensor("bqkv", [3, 128, 1], F32, kind="ExternalInput")
    # wo2: [128 c(2 heads), 1024 o]
    wo2 = nc.dram_tensor("wo2", [128, C], F32R, kind="ExternalInput")
    bo8 = nc.dram_tensor("bo8", [128, C], F32, kind="ExternalInput")
    ident_d = nc.dram_tensor("ident_d", [128, 128], F32R, kind="ExternalInput")
    onesb_d = nc.dram_tensor("onesb_d", [128, 8], BF16, kind="ExternalInput")
    out = nc.dram_tensor("out", [B * L, C], F32, kind="ExternalOutput")

    with tile.TileContext(nc) as tc:
        kernel_body(nc, tc, xT, wqkv, bqkv, wo2, bo8, ident_d,
                    onesb_d, out)
    nc.compile()
    return nc


def kernel_body(nc, tc, xT, wqkv, bqkv, wo2, bo8, ident_d, onesb_d,
                out):
    from contextlib import ExitStack
    ctx = ExitStack()
    with ctx:
        consts = ctx.enter_context(tc.tile_pool(name="consts", bufs=1))
        xpool = ctx.enter_context(tc.tile_pool(name="xpool", bufs=20))
        qkvpool = ctx.enter_context(tc.tile_pool(name="qkvpool", bufs=2))
        vppool = ctx.enter_context(tc.tile_pool(name="vppool", bufs=24))
        epool = ctx.enter_context(tc.tile_pool(name="epool", bufs=18))
        cpool = ctx.enter_context(tc.tile_pool(name="cpool", bufs=2))
        spool = ctx.enter_context(tc.tile_pool(name="spool", bufs=3))
        opool = ctx.enter_context(tc.tile_pool(name="opool", bufs=4))
        # PSUM banks: s-tiles 2x2 + cacc 2 + general 2 = 8
        spsum = ctx.enter_context(tc.tile_pool(name="spsum", bufs=2,
                                               space="PSUM"))
        cpsum = ctx.enter_context(tc.tile_pool(name="cpsum", bufs=2,
                                               space="PSUM"))
        gpsum = ctx.enter_context(tc.tile_pool(name="gpsum", bufs=2,
                                               space="PSUM"))

        # ---- constants ----
        w_tiles = []
        for ci in range(NCT):
            row = []
            for j in range(3):
                t = consts.tile([128, 128], F32R, tag=f"w{ci}_{j}")
                nc.sync.dma_start(out=t, in_=wqkv[ci, j])
                row.append(t)
            w_tiles.append(row)
        b_tiles = []
        for j in range(3):
            t = consts.tile([128, 1], F32, tag=f"b{j}")
            nc.sync.dma_start(out=t, in_=bqkv[j])
            b_tiles.append(t)
        wo_t = consts.tile([128, C], F32R, tag="wo_t")
        nc.sync.dma_start(out=wo_t, in_=wo2[:])
        bias_bc = consts.tile([128, C], F32, tag="bias_bc")
        nc.sync.dma_start(out=bias_bc, in_=bo8[:])
        ident = consts.tile([128, 128], F32R, tag="ident")
        nc.sync.dma_start(out=ident, in_=ident_d[:])
        onesb = consts.tile([128, 8], BF16, tag="onesb")
        nc.sync.dma_start(out=onesb, in_=onesb_d[:])

        for b in range(B):
            # ---- phase 1: qkv projection (channel-major) ----
            # qkvT[j] = [128 f, L]; f packs both heads (h0 at 0:64, h1 64:128)
            qkvT = [qkvpool.tile([128, L], F32R, tag=f"qkvT{j}", name=f"qkvT{j}")
                    for j in range(3)]
            for lc in range(NLC):
                ls = bass.ts(lc, LCHUNK)
                xts = []
                for ci in range(NCT):
                    xt = xpool.tile([128, LCHUNK], F32R, tag="xt")
                    nc.sync.dma_start(out=xt, in_=xT[b, bass.ts(ci, 128), ls])
                    xts.append(xt)
                for j in range(3):
                    p = gpsum.tile([128, LCHUNK], F32, tag="gpb")
                    for ci in range(NCT):
                        nc.tensor.matmul(p, w_tiles[ci][j][:], xts[ci][:],
                                         start=(ci == 0), stop=(ci == NCT - 1))
                    # PSUM -> SBUF with per-partition bias add
                    nc.vector.tensor_scalar_add(qkvT[j][:, ls], p, b_tiles[j][:])

            # ---- phase 1b: v -> token-major bf16 [128 l, v_h0|1|v_h1|1] ----
            vplus = []
            for t in range(NKT):
                tp = gpsum.tile([128, 128], F32R, tag="gpb")
                nc.tensor.transpose(tp, qkvT[2][:, bass.ts(t, 128)], ident[:])
                vp = vppool.tile([128, 2 * HD + 2], BF16, tag="vp")
                nc.vector.tensor_copy(vp[:, 0:HD], tp[:, 0:HD])
                nc.vector.tensor_copy(vp[:, HD + 1:2 * HD + 1], tp[:, HD:2 * HD])
                nc.vector.tensor_copy(vp[:, HD:HD + 1], onesb[:, 0:1])
                nc.vector.tensor_copy(vp[:, 2 * HD + 1:2 * HD + 2], onesb[:, 0:1])
                vplus.append(vp)

            # ---- phase 2: attention per head / q-chunk ----
            # ctxT2 packs both heads: rows 0:64 = h0 channels, 64:128 = h1
            ctxT2 = cpool.tile([128, L], F32R, tag="ctxT2", name="ctxT2")
            for h in range(HPC):
                hb = h * HD  # partition base of this head's q/k channels
                vsl = slice(h * (HD + 1), (h + 1) * (HD + 1))
                for qc in range(2):  # q processed in chunks of 1024
                    q0 = qc * 1024
                    caccs = [cpsum.tile([HD + 1, LCHUNK], F32, tag="cpb",
                                        name=f"cacc{half}")
                             for half in range(2)]
                    evec = []
                    for i in range(NKT):
                        s = spsum.tile([128, 2 * LCHUNK], F32, tag="spb")
                        for half in range(2):
                            nc.tensor.matmul(
                                s[:, bass.ts(half, LCHUNK)],
                                qkvT[1][hb:hb + HD, bass.ts(i, 128)],
                                qkvT[0][hb:hb + HD,
                                        bass.ds(q0 + half * LCHUNK, LCHUNK)],
                                start=True, stop=True)
                        e = epool.tile([128, 2 * LCHUNK], BF16, tag="e")
                        nc.scalar.activation(e, s, EXP, scale=0.125)
                        evec.append(e)
                    for i in range(NKT):
                        for half in range(2):
                            nc.tensor.matmul(
                                caccs[half],
                                vplus[i][:, vsl],
                                evec[i][:, bass.ts(half, LCHUNK)],
                                start=(i == 0), stop=(i == NKT - 1))
                    # normalize: copy cacc out early (frees the PSUM bank),
                    # recip rowsum onto partition 0, GpSimd-broadcast, mul
                    for half in range(2):
                        cacc = caccs[half]
                        qs = bass.ds(q0 + half * LCHUNK, LCHUNK)
                        csb = spool.tile([HD + 1, LCHUNK], F32, tag="csb")
                        nc.vector.tensor_copy(csb, cacc)
                        rt0 = spool.tile([1, LCHUNK], F32, tag="rt0")
                        nc.vector.reciprocal(rt0[0:1, :], csb[HD:HD + 1, :])
                        zs = spool.tile([HD, LCHUNK], F32, tag="zs")
                        nc.gpsimd.partition_broadcast(zs[0:HD, :], rt0[0:1, :])
                        nc.vector.tensor_mul(ctxT2[hb:hb + HD, qs],
                                             csb[0:HD, :], zs)

            # ---- phase 3: output projection (both heads, K=128) ----
            for t in range(NKT):
                rows = bass.ds(b * L + t * 128, 128)
                for oc in range(C // 512):
                    os_ = bass.ts(oc, 512)
                    o = gpsum.tile([128, 512], F32, tag="gpb")
                    nc.tensor.matmul(o, ctxT2[:, bass.ts(t, 128)],
                                     wo_t[:, os_], start=True, stop=True)
                    ot = opool.tile([128, 512], F32, tag="ot")
                    nc.vector.tensor_add(ot, o, bias_bc[:, os_])
                    nc.sync.dma_start(out=out[rows, os_], in_=ot)


_NC_CACHE = None


def get_nc():
    global _NC_CACHE
    if _NC_CACHE is None:
        _NC_CACHE = build_kernel()
    return _NC_CACHE


def prepare_in_maps(x, W_qkv, b_qkv, W_out, b_out):
    x = np.ascontiguousarray(np.asarray(x, np.float32))
    W_qkv = np.asarray(W_qkv, np.float32)
    b_qkv = np.asarray(b_qkv, np.float32)
    W_out = np.asarray(W_out, np.float32)
    b_out = np.asarray(b_out, np.float32)

    xT = np.ascontiguousarray(x.transpose(0, 2, 1))  # [B, C, L]

    in_maps = []
    for core in range(NCORES):
        h0 = HPC * core
        # per-head channel rows in W_qkv: q = h*192..+64, k = +64, v = +128
        qrows = [np.arange(h * 192, h * 192 + 64) for h in (h0, h0 + 1)]
        krows = [q + 64 for q in qrows]
        vrows = [q + 128 for q in qrows]
        fq = np.concatenate(qrows)
        fk = np.concatenate(krows)
        fv = np.concatenate(vrows)
        # wqkv tiles: [ci, j, 128 c, 128 f]
        wt = np.empty((NCT, 3, 128, 128), np.float32)
        for j, rows in enumerate((fq, fk, fv)):
            wT = np.ascontiguousarray(W_qkv[rows].T)  # [1024 c, 128 f]
            wt[:, j] = wT.reshape(NCT, 128, 128)
        bq = np.stack([b_qkv[fq], b_qkv[fk], b_qkv[fv]])[..., None]  # [3,128,1]
        # wo2 = [128 c, 1024 o]: rows 0:64 h0 ctx channels, 64:128 h1
        wo2 = np.concatenate([
            np.ascontiguousarray(W_out[:, (h0 + h) * HD:(h0 + h + 1) * HD].T)
            for h in range(HPC)
        ], axis=0)
        bo8 = np.broadcast_to((b_out / NCORES)[None, :], (128, C))
        in_maps.append({
            "xT": xT,
            "wqkv": wt,
            "bqkv": np.ascontiguousarray(bq),
            "wo2": np.ascontiguousarray(wo2, dtype=np.float32),
            "bo8": np.ascontiguousarray(bo8, dtype=np.float32),
            "ident_d": np.eye(128, dtype=np.float32),
            "onesb_d": np.ones((128, 8), ml_dtypes.bfloat16),
        })
    return in_maps


def kernel(x, W_qkv, b_qkv, W_out, b_out):
    in_maps = prepare_in_maps(x, W_qkv, b_qkv, W_out, b_out)
    res = run_bass_kernel_spmd(get_nc(), in_maps, core_ids=list(range(NCORES)))
    acc = np.zeros((B * L, C), np.float64)
    for core_out in res.results:
        acc += core_out["out"]
    return acc.reshape(B, L, C).astype(np.float32)


if __name__ == "__main__":
    rng = np.random.default_rng(0)
    ins = {
        "x": rng.standard_normal((B, L, C)).astype(np.float32),
        "W_qkv": rng.uniform(-1 / 32, 1 / 32, (3 * C, C)).astype(np.float32),
        "b_qkv": rng.uniform(-1 / 32, 1 / 32, (3 * C,)).astype(np.float32),
        "W_out": rng.uniform(-1 / 32, 1 / 32, (C, C)).astype(np.float32),
        "b_out": rng.uniform(-1 / 32, 1 / 32, (C,)).astype(np.float32),
    }
    o = kernel(**ins)
    print(o.shape, o.dtype)
